# revision 14
# baseline (speedup 1.0000x reference)
"""AGThenGemm, data-parallel: shard B across 8 cores, replicate weights, NO collectives.

Rationale (measured): collectives permanently downshift the PE clock ~20% for the
whole NEFF, so data-parallel (43 GFLOP/core, zero collectives) wins. Weights
stream at ~293 GB/s, under the ~358 GB/s per-core share.

Per core r (B_LOCAL = B/8 = 256):
  GEMM1: act_T[D, B_LOCAL] = W_prev^T @ A_local^T   (A pre-transposed on host)
         -> act_T kept RESIDENT in SBUF, never touches DRAM.
         D rows 0..3071 drained to fp16; rows 3072..4095 drained to fp8e4
         (x32 scale folded into W_prev columns host-side).
  GEMM2: out[B_LOCAL, F] = act_T^T @ W_up, streamed over F blocks of 512.
         Contraction split: 3072 rows fp16 matmuls + 1024 rows fp8 DoubleRow
         matmuls (2x PE throughput). fp8 partial sum carries scale 2^16
         (32 * 2048); combined as out = ps16 + ps8 * 2^-16 (ACT descale ->
         DVE add). Measured rel err 1.88e-2 vs the 2e-2 gate.

All weights are host-repacked so each DMA reads 4KB contiguous per partition.
"""

from contextlib import ExitStack

import numpy as np
import ml_dtypes

import concourse.tile as tile
from concourse import bacc, mybir
from concourse.bass_utils import run_bass_kernel_spmd

N_CORES = 8
B, K_PREV, D, F = 2048, 4096, 4096, 16384
B_LOCAL = B // N_CORES

P = 128
DS = 3072          # fp16 D-rows in GEMM2 contraction
D8 = D - DS        # fp8 D-rows (DoubleRow)
PAIRS = D8 // 256  # DoubleRow instructions per (F-block, mi)
ACT_SCALE = 32.0   # act8 = e4m3(act * 32); folded into W_prev cols DS..D-1
WU8_SCALE = 2048.0
COMBINE = 1.0 / (ACT_SCALE * WU8_SCALE)

F8 = ml_dtypes.float8_e4m3


def build_nc(debug=False):
    nc = bacc.Bacc(
        "TRN2",
        target_bir_lowering=False,
        debug=debug,
        num_devices=N_CORES,
    )
    dt = mybir.dt.float16
    f8 = mybir.dt.float8e4
    f32 = mybir.dt.float32

    KT1 = K_PREV // 512   # 8 k-tiles for GEMM1
    M1_TILES = D // 512   # 8 output-D tiles for GEMM1
    M1_SUB = 4            # 128-subtiles per 512 tile
    NBLKS = F // 512      # 32 F-blocks for GEMM2
    KT2 = DS // 512       # 6 fp16 k-tiles per F-block
    B_SUB = B_LOCAL // P  # 2

    # Host-repacked inputs (see run() for layouts)
    a_t = nc.dram_tensor("a_t", [KT1 * P, 4 * B_LOCAL], dt, kind="ExternalInput")
    w_prev = nc.dram_tensor("w_prev", [M1_TILES * KT1 * P, 4 * 512], dt, kind="ExternalInput")
    wu16 = nc.dram_tensor("wu16", [NBLKS * P, KT2 * 4 * 512], dt, kind="ExternalInput")
    wu8 = nc.dram_tensor("wu8", [NBLKS * P, PAIRS * 2 * 512], f8, kind="ExternalInput")
    out = nc.dram_tensor("out", [B_LOCAL, F], dt, kind="ExternalOutput")

    a4 = a_t.ap().rearrange("(j p) (ki b) -> j p ki b", j=KT1, ki=4)
    wp5 = w_prev.ap().rearrange(
        "(mt kt p) (ki m) -> mt kt p ki m", mt=M1_TILES, kt=KT1, ki=4
    )
    wu16_5 = wu16.ap().rearrange(
        "(nb p) (kt ki n) -> nb p kt ki n", nb=NBLKS, kt=KT2, ki=4
    )
    wu8_5 = wu8.ap().rearrange(
        "(nb p) (pr two n) -> nb p pr two n", nb=NBLKS, pr=PAIRS, two=2
    )
    out3 = out.ap().rearrange("(mo p) n -> p mo n", p=P)  # [P, B_SUB, F]

    with tile.TileContext(nc) as tc:
        with ExitStack() as ctx:
            wp_pool = ctx.enter_context(tc.tile_pool(name="wp_pool", bufs=10))
            wu_pool = ctx.enter_context(tc.tile_pool(name="wu_pool", bufs=12))
            wu8_pool = ctx.enter_context(tc.tile_pool(name="wu8_pool", bufs=4))
            temps = ctx.enter_context(tc.tile_pool(name="temps", bufs=3))
            t8_pool = ctx.enter_context(tc.tile_pool(name="t8_pool", bufs=2))
            res_pool = ctx.enter_context(tc.tile_pool(name="res_pool", bufs=1))
            psum = ctx.enter_context(tc.tile_pool(name="psum", bufs=2, space="PSUM"))

            a_res = res_pool.tile([P, KT1, 4, B_LOCAL], dt, name="a_res")
            act_res = res_pool.tile([P, DS // P, B_LOCAL], dt, name="act_res")
            act8_res = res_pool.tile([P, PAIRS, 2, B_LOCAL], f8, name="act8_res")

            # A^T loads interleave with W_prev panels on the Sync queue —
            # in-order issue self-paces the two streams (measured better than
            # racing them on separate queues).
            nc.sync.dma_start(a_res[:, 0, 0:1, :], a4[0, :, 0:1, :])
            nc.sync.dma_start(a_res[:, 0, 1:4, :], a4[0, :, 1:4, :])

            # GEMM1: act_T = W_prev^T @ A^T, evicted straight into SBUF.
            for mt in range(M1_TILES):
                ps = [
                    psum.tile([P, 512], f32, name=f"ps{mi}", tag=f"ps{mi}")[
                        :, :B_LOCAL
                    ]
                    for mi in range(M1_SUB)
                ]
                for kt in range(KT1):
                    if mt == 0 and kt == 0:
                        # split the very first weight panel so the PE can
                        # start after 256KB instead of 512KB
                        wp_t = wp_pool.tile([P, 4, 512], dt, name="wp_t", tag="wp_t")
                        nc.sync.dma_start(wp_t[:, 0:2, :], wp5[0, 0][:, 0:2, :])
                        nc.sync.dma_start(wp_t[:, 2:4, :], wp5[0, 0][:, 2:4, :])
                    else:
                        wp_t = wp_pool.tile([P, 4, 512], dt, name="wp_t", tag="wp_t")
                        nc.sync.dma_start(wp_t[:], wp5[mt, kt])
                    if mt == 0 and kt + 1 < KT1:
                        j = kt + 1
                        nc.sync.dma_start(a_res[:, j, :, :], a4[j])
                    for ki in range(4):
                        for mi in range(M1_SUB):
                            nc.tensor.matmul(
                                ps[mi][:],
                                wp_t[:, ki, mi * P : (mi + 1) * P],
                                a_res[:, kt, ki, :],
                                start=(kt == 0 and ki == 0),
                                stop=(kt == KT1 - 1 and ki == 3),
                            )
                for mi in range(M1_SUB):
                    gs = mt * M1_SUB + mi  # global 128-subtile index in D
                    if gs < DS // P:
                        nc.vector.tensor_copy(act_res[:, gs, :], ps[mi][:])
                    else:
                        s = gs - DS // P
                        nc.vector.tensor_copy(
                            act8_res[:, s // 2, s % 2, :], ps[mi][:]
                        )

            # GEMM2: out = act_T^T @ W_up. fp8 DoubleRow group first (its ACT
            # descale runs while the fp16 matmuls proceed), then fp16 group.
            for nb in range(NBLKS):
                ps8 = [
                    psum.tile([P, 512], f32, name=f"ps8_{mi}", tag=f"ps{mi}")
                    for mi in range(B_SUB)
                ]
                ps16 = [
                    psum.tile([P, 512], f32, name=f"ps16_{mi}", tag=f"ps{2 + mi}")
                    for mi in range(B_SUB)
                ]
                wu8_t = wu8_pool.tile([P, PAIRS, 2, 512], f8, name="wu8_t", tag="wu8_t")
                # wu8 rides the Scalar queue, parallel to the wu16 stream on Sync.
                nc.scalar.dma_start(wu8_t[:], wu8_5[nb])
                for pr in range(PAIRS):
                    for mi in range(B_SUB):
                        nc.tensor.matmul(
                            ps8[mi][:],
                            act8_res[:, pr, :, mi * P : (mi + 1) * P],
                            wu8_t[:, pr],
                            start=(pr == 0),
                            stop=(pr == PAIRS - 1),
                            perf_mode=mybir.MatmulPerfMode.DoubleRow,
                        )
                t8 = t8_pool.tile([P, B_SUB, 512], f32, name="t8", tag="t8")
                for mi in range(B_SUB):
                    nc.scalar.mul(t8[:, mi], ps8[mi][:], COMBINE)
                for kt in range(KT2):
                    wu_t = wu_pool.tile([P, 4, 512], dt, name="wu_t", tag="wu_t")
                    nc.sync.dma_start(wu_t[:], wu16_5[nb][:, kt])
                    for ki in range(4):
                        for mi in range(B_SUB):
                            nc.tensor.matmul(
                                ps16[mi][:],
                                act_res[:, kt * 4 + ki, mi * P : (mi + 1) * P],
                                wu_t[:, ki],
                                start=(kt == 0 and ki == 0),
                                stop=(kt == KT2 - 1 and ki == 3),
                            )
                ot = temps.tile([P, B_SUB, 512], dt, name="ot", tag="ot")
                for mi in range(B_SUB):
                    nc.vector.tensor_tensor(
                        ot[:, mi], t8[:, mi], ps16[mi][:], mybir.AluOpType.add
                    )
                # Output goes out on the GpSimd queue: it is gated on the DVE
                # combine, and putting it on the Sync queue would serialize the
                # next block's weight prefetch behind it.
                nc.gpsimd.dma_start(out3[:, :, nb * 512 : (nb + 1) * 512], ot[:])
    nc.compile()
    return nc


_NC_CACHE = {}


def _get_nc():
    if "nc" not in _NC_CACHE:
        _NC_CACHE["nc"] = build_nc()
    return _NC_CACHE["nc"]


def _pack_weights(W_prev, W_up):
    KT1 = K_PREV // 512
    M1_TILES = D // 512
    NBLKS = F // 512
    KT2 = DS // 512

    # W_prev: scale cols DS.. by ACT_SCALE (exact pow2 in fp16), pack so each
    # (mt, kt) panel is 4KB contiguous per partition:
    # packed[mt*KT1*P + kt*P + p, ki*512 + m] = Wp[(kt*4+ki)*128+p, mt*512+m]
    Wp = W_prev.copy()
    Wp[:, DS:] *= np.float16(ACT_SCALE)
    wp = (
        Wp.reshape(KT1, 4, P, M1_TILES, 512)
        .transpose(3, 0, 2, 1, 4)
        .reshape(M1_TILES * KT1 * P, 4 * 512)
    )
    wp = np.ascontiguousarray(wp)

    # wu16: packed[nb*P + p, kt*2048 + ki*512 + n] = Wu[(kt*4+ki)*128+p, nb*512+n]
    wu16 = (
        W_up[:DS]
        .reshape(KT2, 4, P, NBLKS, 512)
        .transpose(3, 2, 0, 1, 4)
        .reshape(NBLKS * P, KT2 * 4 * 512)
    )
    wu16 = np.ascontiguousarray(wu16)

    # wu8: e4m3(Wu[DS:] * WU8_SCALE);
    # packed[nb*P + p, pr*1024 + two*512 + n] = q8[(pr*2+two)*128+p, nb*512+n]
    q8 = (W_up[DS:] * np.float16(WU8_SCALE)).astype(F8)
    wu8 = (
        q8.reshape(PAIRS, 2, P, NBLKS, 512)
        .transpose(3, 2, 0, 1, 4)
        .reshape(NBLKS * P, PAIRS * 2 * 512)
    )
    wu8 = np.ascontiguousarray(wu8)
    return wp, wu16, wu8


def run(A_prev, W_prev, W_up, **spmd_kwargs):
    A_prev = np.asarray(A_prev, dtype=np.float16)
    W_prev = np.asarray(W_prev, dtype=np.float16)
    W_up = np.asarray(W_up, dtype=np.float16)
    wp, wu16, wu8 = _pack_weights(W_prev, W_up)
    KT1 = K_PREV // 512
    in_maps = []
    for r in range(N_CORES):
        a_loc = A_prev[r * B_LOCAL : (r + 1) * B_LOCAL, :].T  # [K_PREV, B_LOCAL]
        # packed[j*P + p, ki*B_LOCAL + b] = a_loc[(j*4+ki)*128+p, b]
        a_pk = np.ascontiguousarray(
            a_loc.reshape(KT1, 4, P, B_LOCAL)
            .transpose(0, 2, 1, 3)
            .reshape(KT1 * P, 4 * B_LOCAL)
        )
        in_maps.append({"a_t": a_pk, "w_prev": wp, "wu16": wu16, "wu8": wu8})
    nc = _get_nc()
    res = run_bass_kernel_spmd(
        nc, in_maps, core_ids=list(range(N_CORES)), **spmd_kwargs
    )
    out = np.concatenate([res.results[r]["out"] for r in range(N_CORES)], axis=0)
    return out, res


def kernel(A_prev, W_prev, W_up):
    return run(A_prev, W_prev, W_up)[0]
